# revision 1
# baseline (speedup 1.0000x reference)
import sys
sys.path.insert(0, "/opt/trn_rl_repo")
import numpy as np
import ml_dtypes
import concourse.bacc as bacc
import concourse.bass as bass
import concourse.mybir as mybir
import concourse.tile as tile
from concourse.bass import ds, ts
from concourse.bass_utils import run_bass_kernel_spmd

BF = ml_dtypes.bfloat16
P = 128
NT = 577          # tokens
D = 768
H = 16
HD = 48
KO = 7            # 896 = 7*128 contraction tiles (768 x-dims + bias row + pad)
NBLK = [(0, 128), (128, 128), (256, 128), (384, 128), (512, 65)]
MPAD = 580        # m padded to 4*145
NG = 145          # m4 groups
SCALE = HD ** -0.5

_cache = {}
ABLATE = set()


def _build(nc):
    bf = mybir.dt.bfloat16
    f32 = mybir.dt.float32
    x_d = nc.dram_tensor("xT", [896, NT], bf, kind="ExternalInput")
    xf_d = nc.dram_tensor("xfT", [896, NT], bf, kind="ExternalInput")
    wq_d = nc.dram_tensor("wqT", [896, 1024], bf, kind="ExternalInput")
    wk_d = nc.dram_tensor("wkT", [896, 1024], bf, kind="ExternalInput")
    wv_d = nc.dram_tensor("wvT", [896, 16 * 49], bf, kind="ExternalInput")
    pw_d = nc.dram_tensor("pwT", [1152, D], bf, kind="ExternalInput")
    wb_d = nc.dram_tensor("wblk", [P, 64], bf, kind="ExternalInput")
    id_d = nc.dram_tensor("idn", [P, P], bf, kind="ExternalInput")
    out_d = nc.dram_tensor("outT", [D, NT], f32, kind="ExternalOutput")

    with tile.TileContext(nc) as tc:
        with tc.tile_pool(name="wpool", bufs=1) as wp, \
             tc.tile_pool(name="xw", bufs=2) as xw, \
             tc.tile_pool(name="qk", bufs=1) as qk, \
             tc.tile_pool(name="big", bufs=1) as big, \
             tc.tile_pool(name="sc", bufs=3) as sc, \
             tc.tile_pool(name="ps", bufs=2, space="PSUM") as ps, \
             tc.tile_pool(name="ps1", bufs=1, space="PSUM") as ps1, \
             tc.tile_pool(name="ps2", bufs=2, space="PSUM") as ps2, \
             tc.tile_pool(name="dr", bufs=3, space="DRAM") as dr:

            wq = wp.tile([P, KO, 1024], bf)
            wk = wp.tile([P, KO, 1024], bf)
            wblk = wp.tile([P, 64], bf)
            idn = wp.tile([P, P], bf)
            nc.sync.dma_start(wq[:], wq_d.rearrange("(ko p) m -> p ko m", p=P))
            nc.sync.dma_start(wk[:], wk_d.rearrange("(ko p) m -> p ko m", p=P))
            nc.sync.dma_start(wblk[:], wb_d[:])
            nc.sync.dma_start(idn[:], id_d[:])
            x_sb = xw.tile([P, 9, D], bf, tag="xw")
            xf_sb = xw.tile([P, 9, D], bf, tag="xw")
            nc.sync.dma_start(x_sb[:, :KO, :NT], x_d.rearrange("(ko p) m -> p ko m", p=P))
            nc.sync.dma_start(xf_sb[:, :KO, :NT], xf_d.rearrange("(ko p) m -> p ko m", p=P))
            wv = big.tile([P, KO, 784], bf, tag="E2")  # shares slot with E2 later
            nc.sync.dma_start(wv[:], wv_d.rearrange("(ko p) m -> p ko m", p=P))

            # ---- qkv ----
            qt = qk.tile([P, 8, NT], bf)
            kt = qk.tile([P, 8, NT], bf)
            qft = qk.tile([P, 8, NT], bf)
            kft = qk.tile([P, 8, NT], bf)
            for dst, w_sb, src in () if 'qkv' in ABLATE else ((qt, wq, x_sb), (kt, wk, x_sb), (qft, wq, xf_sb), (kft, wk, xf_sb)):
                for t in range(8):
                    pp = ps.tile([P, 784], f32, tag="big2")
                    for ko in range(KO):
                        for o0, w in ((0, 512), (512, 65)):
                            nc.tensor.matmul(pp[:, o0:o0 + w], w_sb[:, ko, ts(t, P)],
                                             src[:, ko, ds(o0, w)],
                                             start=(ko == 0), stop=(ko == KO - 1))
                    nc.any.tensor_copy(dst[:, t, :], pp[:, :NT])
            # v_aug [tokens, 784]
            wv_v = wv
            v_sb = qk.tile([P, 5, 784], bf)
            for mt, (m0, mlen) in enumerate(NBLK):
                pp = ps.tile([P, 784], f32, tag="big2")
                for ko in range(KO):
                    for o0, w in ((0, 512), (512, 272)):
                        nc.tensor.matmul(pp[:mlen, o0:o0 + w], x_sb[:, ko, ds(m0, mlen)],
                                         wv_v[:, ko, ds(o0, w)], start=(ko == 0), stop=(ko == KO - 1))
                nc.any.tensor_copy(v_sb[:mlen, mt, :], pp[:mlen, :])

            # weighted^T padded [1152, 640] as [128, 9, 640]
            wt = qk.tile([P, 9, 640], bf)
            nc.vector.memset(wt[:], 0.0)
            nc.vector.memset(wt[0:1, 8, :], 1.0)

            pw = xw.tile([P, 9, D], bf, tag="xw")  # reuses x slot after last read
            nc.sync.dma_start(pw[:], pw_d.rearrange("(ko p) m -> p ko m", p=P))

            for n0, nlen in NBLK:
                A = big.tile([P, MPAD, 32], bf, tag="A")
                nc.vector.memset(A[:, NT:MPAD, :], 0.0)
                zt = sc.tile([P, 32], f32, tag="zt")
                zi = sc.tile([P, 32], f32, tag="zi")
                nc.vector.memset(zt[:], 1.0)
                for c in ([] if 'score' in ABLATE else range(32)):
                    qs, ks = (qt, kt) if c < 16 else (qft, kft)
                    h = c % 16
                    t, off = h // 2, 64 * (h % 2)
                    sp = ps.tile([P, 784], f32, tag="big2")
                    for o0, w in ((0, 512), (512, 65)):
                        nc.tensor.matmul(sp[:nlen, o0:o0 + w], qs[off:off + 64, t, ds(n0, nlen)],
                                         ks[off:off + 64, t, ds(o0, w)], start=True, stop=True)
                    nc.scalar.activation(A[:nlen, :NT, c], sp[:nlen, :NT],
                                         mybir.ActivationFunctionType.Exp, scale=SCALE,
                                         accum_out=zt[:nlen, c:c + 1])
                nc.vector.reciprocal(zi[:], zt[:])
                for c in range(32):
                    nc.gpsimd.tensor_scalar_mul(A[:, :NT, c], A[:, :NT, c], zi[:, c:c + 1])

                E2 = big.tile([P, 16, 640], bf, tag="E2")
                nc.vector.memset(E2[:, :, MPAD:640], 0.0)
                E2v = E2[:, :, :MPAD].rearrange("p o (g mj) -> p g mj o", mj=4)
                for gb in ([] if 'mix' in ABLATE else range(0, NG, 4)):
                    ng = min(4, NG - gb)
                    rp = ps2.tile([P, 4, P], bf, tag="tbuf")
                    for gi in range(ng):
                        slab = A[:, ds(4 * (gb + gi), 4), :].rearrange("p m c -> p (m c)")
                        nc.tensor.transpose(rp[:, gi, :], slab, idn[:])
                    rs = sc.tile([P, 4, P], bf, tag="rsb")
                    nc.any.tensor_copy(rs[:, :ng, :], rp[:, :ng, :])
                    mp = ps1.tile([P, 4, 64], f32, tag="m2ps")
                    for gi in range(ng):
                        nc.tensor.matmul(mp[:, gi, :], rs[:, gi, :], wblk[:], start=True, stop=True)
                    mpv = mp.rearrange("p g (mj o) -> p g mj o", o=16)
                    nc.scalar.activation(E2v[:, ds(gb, ng), :, :], mpv[:, :ng, :, :],
                                         mybir.ActivationFunctionType.Exp)

                for o in ([] if 'pv' in ABLATE else range(16)):
                    e2t = sc.tile([P, 5, P], bf, tag="e2t")
                    for mt in range(5):
                        tpb = ps2.tile([P, 4, P], bf, tag="tbuf")
                        tp = tpb[:, 0, :]
                        slab = E2[:, o, ds(128 * mt, P)]
                        nc.tensor.transpose(tp[:, :], slab, idn[:])
                        nc.any.tensor_copy(e2t[:, mt, :], tp[:, :])
                    pv = ps1.tile([P, P], f32, tag="pvps")
                    for mt, (m0, mlen) in enumerate(NBLK):
                        nc.tensor.matmul(pv[:48, :], v_sb[:mlen, mt, ds(49 * o, 48)],
                                         e2t[:mlen, mt, :], start=(mt == 0), stop=(mt == 4))
                        nc.tensor.matmul(pv[64:65, :], v_sb[:mlen, mt, 49 * o + 48:49 * o + 49],
                                         e2t[:mlen, mt, :], start=(mt == 0), stop=(mt == 4))
                    zr = sc.tile([1, P], f32, tag="zri")
                    nc.vector.reciprocal(zr[:], pv[64:65, :])
                    zrep = sc.tile([48, P], f32, tag="zrep")
                    zrd2 = dr.tile([1, P], f32, tag="zrd2")
                    nc.sync.dma_start(zrd2[:], zr[:])
                    bcast = bass.AP(tensor=zrd2.tensor, offset=zrd2.offset,
                                    ap=[[0, 48]] + list(zrd2.ap)[1:])
                    nc.sync.dma_start(zrep[:], bcast)
                    nc.vector.tensor_mul(wt[64 * (o % 2):64 * (o % 2) + 48, o // 2, ds(n0, P)],
                                         pv[0:48, :], zrep[:])

            # ---- proj ----
            for dt in range(6):
                fp = ps.tile([P, 784], f32, tag="big2")
                for ko in range(9):
                    for o0, w in ((0, 512), (512, 65)):
                        nc.tensor.matmul(fp[:, o0:o0 + w], pw[:, ko, ts(dt, P)],
                                         wt[:, ko, ds(o0, w)], start=(ko == 0), stop=(ko == 8))
                ob = sc.tile([P, NT], f32, tag="osb")
                nc.any.tensor_copy(ob[:], fp[:, :NT])
                nc.sync.dma_start(out_d[ts(dt, P), :], ob[:])
    nc.finalize()
    return nc


def _prep_weights(qkv_w, qkv_b, conv_w, proj_w, proj_b):
    f = np.float32
    qkv_w, qkv_b = qkv_w.astype(f), qkv_b.astype(f)
    wq = np.zeros((896, 1024), f)
    wk = np.zeros((896, 1024), f)
    wv = np.zeros((896, 16 * 49), f)
    for h in range(H):
        sl = slice(48 * h, 48 * h + 48)
        wq[:768, 64 * h:64 * h + 48] = qkv_w[sl, :].T
        wq[768, 64 * h:64 * h + 48] = qkv_b[sl]
        wk[:768, 64 * h:64 * h + 48] = qkv_w[768 + 48 * h:768 + 48 * h + 48, :].T
        wk[768, 64 * h:64 * h + 48] = qkv_b[768 + 48 * h:768 + 48 * h + 48]
        wv[:768, 49 * h:49 * h + 48] = qkv_w[1536 + 48 * h:1536 + 48 * h + 48, :].T
        wv[768, 49 * h:49 * h + 48] = qkv_b[1536 + 48 * h:1536 + 48 * h + 48]
        wv[768, 49 * h + 48] = 1.0
    pw = np.zeros((1152, D), f)
    for h in range(H):
        pw[64 * h:64 * h + 48, :] = proj_w.astype(f)[:, 48 * h:48 * h + 48].T
    pw[1024, :] = proj_b.astype(f)
    wblk = np.zeros((128, 64), f)
    for mj in range(4):
        wblk[32 * mj:32 * mj + 32, 16 * mj:16 * mj + 16] = conv_w.astype(f).T
    idn = np.eye(128, dtype=f)
    return {"wqT": wq.astype(BF), "wkT": wk.astype(BF), "wvT": wv.astype(BF),
            "pwT": pw.astype(BF), "wblk": wblk.astype(BF), "idn": idn.astype(BF)}


def kernel(x, x_freq, qkv_w, qkv_b, conv_w, conv_b, proj_w, proj_b, _profile=False):
    # conv_b is constant along the softmax axis -> cancels in softmax; unused.
    if "nc" not in _cache:
        _cache["nc"] = _build(bacc.Bacc())
    nc = _cache["nc"]
    wmap = _prep_weights(np.asarray(qkv_w), np.asarray(qkv_b), np.asarray(conv_w),
                         np.asarray(proj_w), np.asarray(proj_b))
    B = x.shape[0]
    in_maps = []
    for b in range(B):
        xT = np.zeros((896, NT), np.float32)
        xT[:768] = np.asarray(x[b], np.float32).T
        xT[768] = 1.0
        xfT = np.zeros((896, NT), np.float32)
        xfT[:768] = np.asarray(x_freq[b], np.float32).T
        xfT[768] = 1.0
        in_maps.append({"xT": xT.astype(BF), "xfT": xfT.astype(BF), **wmap})
    res = run_bass_kernel_spmd(nc, in_maps, core_ids=list(range(B)), trace=_profile)
    out = np.stack([res.results[b]["outT"].T for b in range(B)], axis=0)
    if _profile:
        return out.astype(np.float32), res
    return out.astype(np.float32)



# revision 12
# speedup vs baseline: 2.8689x; 2.8689x over previous
import sys
sys.path.insert(0, "/opt/trn_rl_repo")
import numpy as np
import ml_dtypes
import concourse.bacc as bacc
import concourse.bass as bass
import concourse.mybir as mybir
import concourse.tile as tile
from concourse.bass import ds, ts
from concourse.bass_utils import run_bass_kernel_spmd

BF = ml_dtypes.bfloat16
P = 128
NT = 577          # tokens
NPAD = 580        # tokens padded to 4*145
NG = 145          # token groups of 4 (for channel-mix transposes)
D = 768
H = 16
HD = 48
KO = 7            # 896 = 7*128 contraction tiles (768 dims + bias row + pad)
NBLK = [(0, 128), (128, 128), (256, 128), (384, 128), (512, 65)]
SCALE = HD ** -0.5

_cache = {}


def _build(nc):
    bf = mybir.dt.bfloat16
    f32 = mybir.dt.float32
    Exp = mybir.ActivationFunctionType.Exp
    Ln = mybir.ActivationFunctionType.Ln

    x_d = nc.dram_tensor("xT", [896, NT], bf, kind="ExternalInput")
    xf_d = nc.dram_tensor("xfT", [896, NT], bf, kind="ExternalInput")
    wqk_d = nc.dram_tensor("wqkT", [896, 2048], bf, kind="ExternalInput")
    wv_d = nc.dram_tensor("wvT", [896, 16 * 49], bf, kind="ExternalInput")
    wb_d = nc.dram_tensor("wblk", [P, 64], bf, kind="ExternalInput")
    id_d = nc.dram_tensor("idn", [P, P], bf, kind="ExternalInput")
    pw_d = nc.dram_tensor("pwT", [896, D], bf, kind="ExternalInput")
    out_d = nc.dram_tensor("outT", [D, NT], f32, kind="ExternalOutput")

    with tile.TileContext(nc) as tc:
        with tc.tile_pool(name="wqk", bufs=1) as wqkp, \
             tc.tile_pool(name="big", bufs=1) as big, \
             tc.tile_pool(name="xw", bufs=2) as xw, \
             tc.tile_pool(name="qk", bufs=1) as qk, \
             tc.tile_pool(name="sc", bufs=3) as sc, \
             tc.tile_pool(name="psB", bufs=2, space="PSUM") as psB, \
             tc.tile_pool(name="psT", bufs=2, space="PSUM") as psT, \
             tc.tile_pool(name="psM", bufs=2, space="PSUM") as psM:

            # ---- load weights & inputs ----
            wqk = wqkp.tile([P, KO, 2048], bf, tag="wqk")
            nc.sync.dma_start(wqk[:], wqk_d.rearrange("(ko p) m -> p ko m", p=P))
            wblk = qk.tile([P, 64], bf)
            idn = qk.tile([P, P], bf)
            nc.sync.dma_start(wblk[:], wb_d[:])
            nc.sync.dma_start(idn[:], id_d[:])
            x_sb = xw.tile([P, KO, NPAD], bf, tag="xw")
            xf_sb = xw.tile([P, KO, NPAD], bf, tag="xw")
            nc.sync.dma_start(x_sb[:, :, :NT], x_d.rearrange("(ko p) m -> p ko m", p=P))
            nc.sync.dma_start(xf_sb[:, :, :NT], xf_d.rearrange("(ko p) m -> p ko m", p=P))
            wv = big.tile([P, KO, 784], bf, tag="E2T")  # slot reused by E2T later
            nc.sync.dma_start(wv[:], wv_d.rearrange("(ko p) m -> p ko m", p=P))

            # ---- qkv: q/k for both branches, [dh, token] orientation ----
            # qt/kt rows per head h: partitions 64*(h%2) .. +47 at t=h//2;
            # row 64*(h%2)+48 is the augmentation slot (ones for k, -ln z for q).
            qt = qk.tile([P, 8, NPAD], bf)
            kt = qk.tile([P, 8, NPAD], bf)
            qft = qk.tile([P, 8, NPAD], bf)
            kft = qk.tile([P, 8, NPAD], bf)
            for dst, wt_, src in ((qt, 0, x_sb), (kt, 1, x_sb), (qft, 0, xf_sb), (kft, 1, xf_sb)):
                for t in range(8):
                    pp = psB.tile([P, 784], f32, tag="big")
                    for ko in range(KO):
                        for o0, w in ((0, 512), (512, 65)):
                            nc.tensor.matmul(pp[:, o0:o0 + w], wqk[:, ko, ds(1024 * wt_ + 128 * t, P)],
                                             src[:, ko, ds(o0, w)],
                                             start=(ko == 0), stop=(ko == KO - 1))
                    nc.vector.tensor_copy(dst[:, t, :NT], pp[:, :NT])

            # ---- v (augmented with ones col per head): [token, 16*49] ----
            v_sb = qk.tile([P, 5, 784], bf)
            for mt, (m0, mlen) in enumerate(NBLK):
                pp = psB.tile([P, 784], f32, tag="big")
                for ko in range(KO):
                    for o0, w in ((0, 512), (512, 272)):
                        nc.tensor.matmul(pp[:mlen, o0:o0 + w], x_sb[:, ko, ds(m0, mlen)],
                                         wv[:, ko, ds(o0, w)], start=(ko == 0), stop=(ko == KO - 1))
                nc.vector.tensor_copy(v_sb[:mlen, mt, :], pp[:mlen, :])

            # ---- pass 1: softmax-1 denominators z[c, n] ----
            zt = qk.tile([P, 5, 32], f32)
            for c in range(32):
                qs, ks = (qt, kt) if c < 16 else (qft, kft)
                h = c % 16
                t, off = h // 2, 64 * (h % 2)
                for bi, (n0, nlen) in enumerate(NBLK):
                    sp = psB.tile([P, 784], f32, tag="big")
                    for o0, w in ((0, 512), (512, 65)):
                        nc.tensor.matmul(sp[:nlen, o0:o0 + w], qs[off:off + 48, t, ds(n0, nlen)],
                                         ks[off:off + 48, t, ds(o0, w)], start=True, stop=True)
                    scr = wqkp.tile([P, NPAD], bf, tag="wqk")
                    nc.scalar.activation(scr[:nlen, :NT], sp[:nlen, :NT], Exp,
                                         accum_out=zt[:nlen, bi, c:c + 1])

            # ---- -ln z, transposed to [c, n] ----
            negLnzT = qk.tile([32, NPAD], bf)
            for bi, (n0, nlen) in enumerate(NBLK):
                lnt = sc.tile([P, 32], bf, tag="lnt")
                nc.scalar.activation(lnt[:nlen, :], zt[:nlen, bi, :], Ln)
                ltp = psT.tile([32, P], bf, tag="tp")
                nc.tensor.transpose(ltp[:, :nlen], lnt[:nlen, :], idn[:nlen, :nlen])
                nc.vector.tensor_scalar_mul(negLnzT[:, ds(n0, nlen)], ltp[:, :nlen], -1.0)

            # scatter -ln z into the q augmentation rows (partition 48/112)
            for dst, c0 in ((qt, 0), (qft, 16)):
                for t in range(8):
                    nc.sync.dma_start(dst[48:49, t, :NT], negLnzT[c0 + 2 * t:c0 + 2 * t + 1, :NT])
                    nc.sync.dma_start(dst[112:113, t, :NT], negLnzT[c0 + 2 * t + 1:c0 + 2 * t + 2, :NT])

            # ---- E2T: per m-block: pass2 scores^T -> normalized A^T -> mix -> exp ----
            E2T = big.tile([P, 5, H, NPAD], bf, tag="E2T")
            for mi, (m0, mlen) in enumerate(NBLK):
                # A^T in slab-interleaved layout: [m, g, (c, nj)] where n = 4g + nj
                AT = wqkp.tile([P, NG, P], bf, tag="wqk")  # reuses wqk slot
                # pass 2: s^T - ln z via augmented K=49 contraction; exp -> A^T
                for c in range(32):
                    qs, ks = (qt, kt) if c < 16 else (qft, kft)
                    h = c % 16
                    t, off = h // 2, 64 * (h % 2)
                    sp = psB.tile([P, 784], f32, tag="big")
                    for o0, w in ((0, 512), (512, 65)):
                        nc.tensor.matmul(sp[:mlen, o0:o0 + w], ks[off:off + 49, t, ds(m0, mlen)],
                                         qs[off:off + 49, t, ds(o0, w)], start=True, stop=True)
                    nc.scalar.activation(AT[:mlen, :, ds(4 * c, 4)],
                                         sp[:mlen, :NPAD].rearrange("p (g m) -> p g m", m=4),
                                         Exp)
                # zero the n-padding lanes (n = 577..579 -> g = 144, nj = 1..3)
                nc.vector.memset(
                    AT[:mlen, NG - 1, :].rearrange("p (c m) -> p c m", m=4)[:, :, 1:4], 0.0)
                # mix: transpose 4-token slabs, matmul with conv block weights, exp
                for gb in range(0, NG, 4):
                    ng = min(4, NG - gb)
                    mp = psM.tile([P, 4, 64], f32, tag="mp")
                    for gi in range(ng):
                        g = gb + gi
                        rp = psT.tile([P, P], bf, tag="tp")
                        nc.tensor.transpose(rp[:, :mlen], AT[:mlen, g, :], idn[:mlen, :mlen])
                        rs = sc.tile([P, P], bf, tag="tmp")
                        nc.vector.tensor_copy(rs[:, :mlen], rp[:, :mlen])
                        nc.tensor.matmul(mp[:mlen, gi, :], rs[:, :mlen], wblk[:],
                                         start=True, stop=True)
                    nc.scalar.activation(
                        E2T[:mlen, mi, :, ds(4 * gb, 4 * ng)].rearrange("p o (g m) -> p g o m", g=ng),
                        mp[:mlen, :ng, :].rearrange("p g (o m) -> p g o m", m=4),
                        Exp)

            # ---- PV: out[n, 48+1] per head, accumulate over m-blocks ----
            wt2 = qk.tile([P, H, HD], bf)
            wtT = qk.tile([P, KO, NPAD], bf)
            nc.vector.memset(wtT[:, 6, :], 0.0)
            nc.vector.memset(wtT[0:1, 6, :], 1.0)
            for ni, (n0, nlen) in enumerate(NBLK):
                for o in range(H):
                    pv = psM.tile([P, 64], f32, tag="mp")
                    for mi, (m0, mlen) in enumerate(NBLK):
                        nc.tensor.matmul(pv[:nlen, :49], E2T[:mlen, mi, o, ds(n0, nlen)],
                                         v_sb[:mlen, mi, ds(49 * o, 49)],
                                         start=(mi == 0), stop=(mi == 4))
                    zi2 = sc.tile([P, 1], f32, tag="zi")
                    nc.vector.reciprocal(zi2[:nlen], pv[:nlen, 48:49])
                    nc.vector.tensor_scalar_mul(wt2[:nlen, o, :], pv[:nlen, :48], zi2[:nlen])
                # transpose weighted [n, 768] -> [768, n] for the projection
                wt2f = wt2.rearrange("p o d -> p (o d)")
                for dt in range(6):
                    tp = psT.tile([P, P], bf, tag="tp")
                    nc.tensor.transpose(tp[:, :nlen], wt2f[:nlen, ts(dt, P)], idn[:nlen, :nlen])
                    nc.vector.tensor_copy(wtT[:, dt, ds(n0, nlen)], tp[:, :nlen])

            # ---- proj ----
            pw = xw.tile([P, KO, D], bf, tag="xw")  # reuses x slot
            nc.sync.dma_start(pw[:], pw_d.rearrange("(ko p) m -> p ko m", p=P))
            for dt in range(6):
                fp = psB.tile([P, 784], f32, tag="big")
                for ko in range(KO):
                    for o0, w in ((0, 512), (512, 65)):
                        nc.tensor.matmul(fp[:, o0:o0 + w], pw[:, ko, ts(dt, P)],
                                         wtT[:, ko, ds(o0, w)], start=(ko == 0), stop=(ko == KO - 1))
                ob = wqkp.tile([P, NT], f32, tag="wqk")
                nc.vector.tensor_copy(ob[:], fp[:, :NT])
                nc.sync.dma_start(out_d[ts(dt, P), :], ob[:])
    nc.finalize()
    return nc


def _prep_weights(qkv_w, qkv_b, conv_w, proj_w, proj_b):
    f = np.float32
    qkv_w, qkv_b = qkv_w.astype(f), qkv_b.astype(f)
    wqk = np.zeros((896, 2048), f)
    wv = np.zeros((896, 16 * 49), f)
    for h in range(H):
        q = slice(48 * h, 48 * h + 48)
        k = slice(768 + 48 * h, 768 + 48 * h + 48)
        v = slice(1536 + 48 * h, 1536 + 48 * h + 48)
        wqk[:768, 64 * h:64 * h + 48] = qkv_w[q, :].T * SCALE
        wqk[768, 64 * h:64 * h + 48] = qkv_b[q] * SCALE
        wqk[:768, 1024 + 64 * h:1024 + 64 * h + 48] = qkv_w[k, :].T
        wqk[768, 1024 + 64 * h:1024 + 64 * h + 48] = qkv_b[k]
        wqk[768, 1024 + 64 * h + 48] = 1.0          # k-tilde ones component
        wv[:768, 49 * h:49 * h + 48] = qkv_w[v, :].T
        wv[768, 49 * h:49 * h + 48] = qkv_b[v]
        wv[768, 49 * h + 48] = 1.0                  # softmax-2 denominator col
    wblk = np.zeros((128, 64), f)
    cw = conv_w.astype(f)
    for c in range(32):
        for nj in range(4):
            wblk[4 * c + nj, nj::4] = cw[:, c]
    pw = np.zeros((896, D), f)
    pw[:768, :] = proj_w.astype(f).T
    pw[768, :] = proj_b.astype(f)
    idn = np.eye(128, dtype=f)
    return {"wqkT": wqk.astype(BF), "wvT": wv.astype(BF), "wblk": wblk.astype(BF),
            "pwT": pw.astype(BF), "idn": idn.astype(BF)}


def kernel(x, x_freq, qkv_w, qkv_b, conv_w, conv_b, proj_w, proj_b, _profile=False):
    # conv_b is constant along the softmax axis -> cancels in softmax; unused.
    if "nc" not in _cache:
        _cache["nc"] = _build(bacc.Bacc())
    nc = _cache["nc"]
    wmap = _prep_weights(np.asarray(qkv_w), np.asarray(qkv_b), np.asarray(conv_w),
                         np.asarray(proj_w), np.asarray(proj_b))
    B = x.shape[0]
    in_maps = []
    for b in range(B):
        xT = np.zeros((896, NT), np.float32)
        xT[:768] = np.asarray(x[b], np.float32).T
        xT[768] = 1.0
        xfT = np.zeros((896, NT), np.float32)
        xfT[:768] = np.asarray(x_freq[b], np.float32).T
        xfT[768] = 1.0
        in_maps.append({"xT": xT.astype(BF), "xfT": xfT.astype(BF), **wmap})
    res = run_bass_kernel_spmd(nc, in_maps, core_ids=list(range(B)), trace=_profile)
    out = np.stack([res.results[b]["outT"].T for b in range(B)], axis=0)
    if _profile:
        return out.astype(np.float32), res
    return out.astype(np.float32)


# revision 14
# speedup vs baseline: 3.2971x; 1.1493x over previous
import sys
sys.path.insert(0, "/opt/trn_rl_repo")
import numpy as np
import ml_dtypes
import concourse.bacc as bacc
import concourse.bass as bass
import concourse.mybir as mybir
import concourse.tile as tile
from concourse.bass import ds, ts
from concourse.bass_utils import run_bass_kernel_spmd

BF = ml_dtypes.bfloat16
P = 128
NT = 577          # tokens
NPAD = 580        # tokens padded to 4*145
NG = 145          # token groups of 4 (for channel-mix transposes)
D = 768
H = 16
HD = 48
KO = 7            # 896 = 7*128 contraction tiles (768 dims + bias row + pad)
NBLK = [(0, 128), (128, 128), (256, 128), (384, 128), (512, 65)]
SCALE = HD ** -0.5

_cache = {}


def _build(nc):
    bf = mybir.dt.bfloat16
    f32 = mybir.dt.float32
    Exp = mybir.ActivationFunctionType.Exp
    Ln = mybir.ActivationFunctionType.Ln

    x_d = nc.dram_tensor("xT", [896, NT], bf, kind="ExternalInput")
    xf_d = nc.dram_tensor("xfT", [896, NT], bf, kind="ExternalInput")
    wqk_d = nc.dram_tensor("wqkT", [896, 2048], bf, kind="ExternalInput")
    wv_d = nc.dram_tensor("wvT", [896, 16 * 49], bf, kind="ExternalInput")
    wb_d = nc.dram_tensor("wblk", [P, 64], bf, kind="ExternalInput")
    id_d = nc.dram_tensor("idn", [P, P], bf, kind="ExternalInput")
    pw_d = nc.dram_tensor("pwT", [896, D], bf, kind="ExternalInput")
    out_d = nc.dram_tensor("outT", [D, NT], f32, kind="ExternalOutput")

    with tile.TileContext(nc) as tc:
        with tc.tile_pool(name="wqk", bufs=1) as wqkp, \
             tc.tile_pool(name="big", bufs=1) as big, \
             tc.tile_pool(name="xw", bufs=2) as xw, \
             tc.tile_pool(name="qk", bufs=1) as qk, \
             tc.tile_pool(name="sc", bufs=3) as sc, \
             tc.tile_pool(name="psB", bufs=2, space="PSUM") as psB, \
             tc.tile_pool(name="psT", bufs=2, space="PSUM") as psT, \
             tc.tile_pool(name="psM", bufs=2, space="PSUM") as psM:

            # ---- load weights & inputs ----
            wqk = wqkp.tile([P, KO, 2048], bf, tag="wqk")
            nc.sync.dma_start(wqk[:], wqk_d.rearrange("(ko p) m -> p ko m", p=P))
            wblk = qk.tile([P, 64], bf)
            idn = qk.tile([P, P], bf)
            nc.sync.dma_start(wblk[:], wb_d[:])
            nc.sync.dma_start(idn[:], id_d[:])
            x_sb = xw.tile([P, KO, NPAD], bf, tag="xw")
            xf_sb = xw.tile([P, KO, NPAD], bf, tag="xw")
            nc.sync.dma_start(x_sb[:, :, :NT], x_d.rearrange("(ko p) m -> p ko m", p=P))
            nc.sync.dma_start(xf_sb[:, :, :NT], xf_d.rearrange("(ko p) m -> p ko m", p=P))
            wv = big.tile([P, KO, 784], bf, tag="E2T")  # slot reused by E2T later
            nc.sync.dma_start(wv[:], wv_d.rearrange("(ko p) m -> p ko m", p=P))

            # ---- qkv: q/k for both branches, [dh, token] orientation ----
            # qt/kt rows per head h: partitions 64*(h%2) .. +47 at t=h//2;
            # row 64*(h%2)+48 is the augmentation slot (ones for k, -ln z for q).
            qt = qk.tile([P, 8, NPAD], bf)
            kt = qk.tile([P, 8, NPAD], bf)
            qft = qk.tile([P, 8, NPAD], bf)
            kft = qk.tile([P, 8, NPAD], bf)
            for dst, wt_, src in ((qt, 0, x_sb), (kt, 1, x_sb), (qft, 0, xf_sb), (kft, 1, xf_sb)):
                for t in range(8):
                    pp = psB.tile([P, 784], f32, tag="big")
                    for ko in range(KO):
                        for o0, w in ((0, 512), (512, 65)):
                            nc.tensor.matmul(pp[:, o0:o0 + w], wqk[:, ko, ds(1024 * wt_ + 128 * t, P)],
                                             src[:, ko, ds(o0, w)],
                                             start=(ko == 0), stop=(ko == KO - 1))
                    nc.vector.tensor_copy(dst[:, t, :NT], pp[:, :NT])

            # ---- v (augmented with ones col per head): [token, 16*49] ----
            v_sb = qk.tile([P, 5, 784], bf)
            for mt, (m0, mlen) in enumerate(NBLK):
                pp = psB.tile([P, 784], f32, tag="big")
                for ko in range(KO):
                    for o0, w in ((0, 512), (512, 272)):
                        nc.tensor.matmul(pp[:mlen, o0:o0 + w], x_sb[:, ko, ds(m0, mlen)],
                                         wv[:, ko, ds(o0, w)], start=(ko == 0), stop=(ko == KO - 1))
                nc.vector.tensor_copy(v_sb[:mlen, mt, :], pp[:mlen, :])

            # ---- pass 1: softmax-1 denominators z[c, n] ----
            zt = qk.tile([P, 5, 32], f32)
            for c in range(32):
                qs, ks = (qt, kt) if c < 16 else (qft, kft)
                h = c % 16
                t, off = h // 2, 64 * (h % 2)
                for bi, (n0, nlen) in enumerate(NBLK):
                    sp = psB.tile([P, 784], f32, tag="big")
                    for o0, w in ((0, 512), (512, 65)):
                        nc.tensor.matmul(sp[:nlen, o0:o0 + w], qs[off:off + 48, t, ds(n0, nlen)],
                                         ks[off:off + 48, t, ds(o0, w)], start=True, stop=True)
                    scr = wqkp.tile([P, NPAD], bf, tag="wqk")
                    nc.scalar.activation(scr[:nlen, :NT], sp[:nlen, :NT], Exp,
                                         accum_out=zt[:nlen, bi, c:c + 1])

            # ---- -ln z, transposed to [c, n] ----
            negLnzT = qk.tile([32, NPAD], bf)
            for bi, (n0, nlen) in enumerate(NBLK):
                lnt = sc.tile([P, 32], bf, tag="lnt")
                nc.scalar.activation(lnt[:nlen, :], zt[:nlen, bi, :], Ln)
                ltp = psT.tile([32, P], bf, tag="tp")
                nc.tensor.transpose(ltp[:, :nlen], lnt[:nlen, :], idn[:nlen, :nlen])
                nc.vector.tensor_scalar_mul(negLnzT[:, ds(n0, nlen)], ltp[:, :nlen], -1.0)

            # scatter -ln z into the q augmentation rows (partition 48/112)
            for dst, c0 in ((qt, 0), (qft, 16)):
                for t in range(8):
                    nc.sync.dma_start(dst[48:49, t, :NT], negLnzT[c0 + 2 * t:c0 + 2 * t + 1, :NT])
                    nc.sync.dma_start(dst[112:113, t, :NT], negLnzT[c0 + 2 * t + 1:c0 + 2 * t + 2, :NT])

            # ---- E2T: per m-block: pass2 scores^T -> normalized A^T -> mix -> exp ----
            E2T = big.tile([P, 5, H, NPAD], bf, tag="E2T")
            for mi, (m0, mlen) in enumerate(NBLK):
                # A^T in slab-interleaved layout: [m, g, (c, nj)] where n = 4g + nj
                AT = wqkp.tile([P, NG, P], bf, tag="wqk")  # reuses wqk slot
                # pass 2: s^T - ln z via augmented K=49 contraction; exp -> A^T
                for c in range(32):
                    qs, ks = (qt, kt) if c < 16 else (qft, kft)
                    h = c % 16
                    t, off = h // 2, 64 * (h % 2)
                    sp = psB.tile([P, 784], f32, tag="big")
                    for o0, w in ((0, 512), (512, 65)):
                        nc.tensor.matmul(sp[:mlen, o0:o0 + w], ks[off:off + 49, t, ds(m0, mlen)],
                                         qs[off:off + 49, t, ds(o0, w)], start=True, stop=True)
                    nc.scalar.activation(AT[:mlen, :, ds(4 * c, 4)],
                                         sp[:mlen, :NPAD].rearrange("p (g m) -> p g m", m=4),
                                         Exp)
                # zero the n-padding lanes (n = 577..579 -> g = 144, nj = 1..3)
                nc.vector.memset(
                    AT[:mlen, NG - 1, :].rearrange("p (c m) -> p c m", m=4)[:, :, 1:4], 0.0)
                # mix: transpose 4-token slabs, matmul with conv block weights, exp
                for gb in range(0, NG, 4):
                    ng = min(4, NG - gb)
                    mp = psM.tile([P, 4, 64], f32, tag="mp")
                    for gi in range(ng):
                        g = gb + gi
                        rp = psT.tile([P, P], f32, tag="tp")
                        nc.tensor.matmul(rp[:, :mlen], AT[:mlen, g, :], idn[:mlen, :mlen],
                                         start=True, stop=True)
                        rs = sc.tile([P, P], bf, tag="tmp")
                        nc.vector.tensor_copy(rs[:, :mlen], rp[:, :mlen])
                        nc.tensor.matmul(mp[:mlen, gi, :], rs[:, :mlen], wblk[:],
                                         start=True, stop=True)
                    nc.scalar.activation(
                        E2T[:mlen, mi, :, ds(4 * gb, 4 * ng)].rearrange("p o (g m) -> p g o m", g=ng),
                        mp[:mlen, :ng, :].rearrange("p g (o m) -> p g o m", m=4),
                        Exp)

            # ---- PV: out[n, 48+1] per head, accumulate over m-blocks ----
            wt2 = qk.tile([P, H, HD], bf)
            wtT = qk.tile([P, KO, NPAD], bf)
            nc.vector.memset(wtT[:, 6, :], 0.0)
            nc.vector.memset(wtT[0:1, 6, :], 1.0)
            for ni, (n0, nlen) in enumerate(NBLK):
                for o in range(H):
                    pv = psM.tile([P, 64], f32, tag="mp")
                    for mi, (m0, mlen) in enumerate(NBLK):
                        nc.tensor.matmul(pv[:nlen, :49], E2T[:mlen, mi, o, ds(n0, nlen)],
                                         v_sb[:mlen, mi, ds(49 * o, 49)],
                                         start=(mi == 0), stop=(mi == 4))
                    zi2 = sc.tile([P, 1], f32, tag="zi")
                    nc.vector.reciprocal(zi2[:nlen], pv[:nlen, 48:49])
                    nc.vector.tensor_scalar_mul(wt2[:nlen, o, :], pv[:nlen, :48], zi2[:nlen])
                # transpose weighted [n, 768] -> [768, n] for the projection
                wt2f = wt2.rearrange("p o d -> p (o d)")
                for dt in range(6):
                    tp = psT.tile([P, P], f32, tag="tp")
                    nc.tensor.matmul(tp[:, :nlen], wt2f[:nlen, ts(dt, P)], idn[:nlen, :nlen],
                                     start=True, stop=True)
                    nc.vector.tensor_copy(wtT[:, dt, ds(n0, nlen)], tp[:, :nlen])

            # ---- proj ----
            pw = xw.tile([P, KO, D], bf, tag="xw")  # reuses x slot
            nc.sync.dma_start(pw[:], pw_d.rearrange("(ko p) m -> p ko m", p=P))
            for dt in range(6):
                fp = psB.tile([P, 784], f32, tag="big")
                for ko in range(KO):
                    for o0, w in ((0, 512), (512, 65)):
                        nc.tensor.matmul(fp[:, o0:o0 + w], pw[:, ko, ts(dt, P)],
                                         wtT[:, ko, ds(o0, w)], start=(ko == 0), stop=(ko == KO - 1))
                ob = wqkp.tile([P, NT], f32, tag="wqk")
                nc.vector.tensor_copy(ob[:], fp[:, :NT])
                nc.sync.dma_start(out_d[ts(dt, P), :], ob[:])
    nc.finalize()
    return nc


def _prep_weights(qkv_w, qkv_b, conv_w, proj_w, proj_b):
    f = np.float32
    qkv_w, qkv_b = qkv_w.astype(f), qkv_b.astype(f)
    wqk = np.zeros((896, 2048), f)
    wv = np.zeros((896, 16 * 49), f)
    for h in range(H):
        q = slice(48 * h, 48 * h + 48)
        k = slice(768 + 48 * h, 768 + 48 * h + 48)
        v = slice(1536 + 48 * h, 1536 + 48 * h + 48)
        wqk[:768, 64 * h:64 * h + 48] = qkv_w[q, :].T * SCALE
        wqk[768, 64 * h:64 * h + 48] = qkv_b[q] * SCALE
        wqk[:768, 1024 + 64 * h:1024 + 64 * h + 48] = qkv_w[k, :].T
        wqk[768, 1024 + 64 * h:1024 + 64 * h + 48] = qkv_b[k]
        wqk[768, 1024 + 64 * h + 48] = 1.0          # k-tilde ones component
        wv[:768, 49 * h:49 * h + 48] = qkv_w[v, :].T
        wv[768, 49 * h:49 * h + 48] = qkv_b[v]
        wv[768, 49 * h + 48] = 1.0                  # softmax-2 denominator col
    wblk = np.zeros((128, 64), f)
    cw = conv_w.astype(f)
    for c in range(32):
        for nj in range(4):
            wblk[4 * c + nj, nj::4] = cw[:, c]
    pw = np.zeros((896, D), f)
    pw[:768, :] = proj_w.astype(f).T
    pw[768, :] = proj_b.astype(f)
    idn = np.eye(128, dtype=f)
    return {"wqkT": wqk.astype(BF), "wvT": wv.astype(BF), "wblk": wblk.astype(BF),
            "pwT": pw.astype(BF), "idn": idn.astype(BF)}


def kernel(x, x_freq, qkv_w, qkv_b, conv_w, conv_b, proj_w, proj_b, _profile=False):
    # conv_b is constant along the softmax axis -> cancels in softmax; unused.
    if "nc" not in _cache:
        _cache["nc"] = _build(bacc.Bacc())
    nc = _cache["nc"]
    wmap = _prep_weights(np.asarray(qkv_w), np.asarray(qkv_b), np.asarray(conv_w),
                         np.asarray(proj_w), np.asarray(proj_b))
    B = x.shape[0]
    in_maps = []
    for b in range(B):
        xT = np.zeros((896, NT), np.float32)
        xT[:768] = np.asarray(x[b], np.float32).T
        xT[768] = 1.0
        xfT = np.zeros((896, NT), np.float32)
        xfT[:768] = np.asarray(x_freq[b], np.float32).T
        xfT[768] = 1.0
        in_maps.append({"xT": xT.astype(BF), "xfT": xfT.astype(BF), **wmap})
    res = run_bass_kernel_spmd(nc, in_maps, core_ids=list(range(B)), trace=_profile)
    out = np.stack([res.results[b]["outT"].T for b in range(B)], axis=0)
    if _profile:
        return out.astype(np.float32), res
    return out.astype(np.float32)
